# revision 4
# baseline (speedup 1.0000x reference)
"""Trainium2 Bass kernel for nn_BestHits: out = bh * bh.T where
bh = blockwise-softmax(mask_diag(similarities) / TAU) over 256-wide column groups.

Strategy: out is symmetric (out.T = bh.T * bh = out), so only the upper
triangle of 512x512 block-pairs is computed on device. The 16x16 block grid
has 136 upper-incl-diagonal pairs = 17 per core on 8 cores (each core gets
exactly 2 diagonal + 15 off-diagonal pairs -> perfectly uniform SPMD work).
B-side blocks are staged pre-transposed by the host (layout-only, free).

v2 redesign (from v1's 142.6us ntff profile: ACT 113.6us busy with 136
ACTIVATION_READ_ACCUMULATORs, DVE 111us busy with 136 1x-only
SCALAR_TENSOR_TENSORs, DMA 104.7us for 42.5MB):

  * Inputs staged fp16 on the host (free) -> 16 MiB instead of 32 MiB of
    loads per core; DMA wall ~105us -> ~63us. Measured numpy rel_fro with
    fp16 inputs: 2.4e-3 against the 2e-2 budget.
  * A-side exp as 2 big ACTIVATEs (no accum_out) instead of 8 small
    accumulating ones: kills 8 READ_ACCUMULATOR (282ns each) + 6 instr
    overheads per slot on the ACT engine.
  * A-side group sums on DVE via tensor_scalar(copy, accum_out=sa) per
    (t,g): InstTensorScalarPtr supports 4x_2p DVE perf mode (0.26ns/elem
    with all-16-bit packed SBUF operands; f32 [P,1] scalars exempt).
  * out = (za*ra)*X split as 8x 4x-mode tensor_scalar_mul (wa = za*ra) +
    ONE 2x-mode tensor_tensor (all-16-bit) instead of 8 1x-only
    scalar_tensor_tensors: ~4.5us -> ~2.2us DVE per slot.
  * X = zbt*rp moved fully to GpSimd (2 ops per slot, rp f32 from SBUF).
  * Output stores issued from the Tensor engine's DMA queue (PE is the
    least busy engine; keeps stores off the load ring AND off GpSimd).

Engine-wall projection per off slot: ACT 4.2us, DVE ~4.7us, GpSimd ~4.3us,
PE ~2.5us, DMA ~3.7us -> ~80us walls vs v1's 111-114us.

Per-core HBM traffic: 15*1 MiB + 2*0.5 MiB loads + 17*0.5 MiB stores
= 24.5 MiB -> ~63us roofline at the ~400 GB/s the trace sustains.
"""
import sys

import numpy as np

sys.path.insert(0, "/opt/trn_rl_repo")

from contextlib import ExitStack

import concourse.bass as bass  # noqa: F401  (registers AP machinery)
import concourse.tile as tile
from concourse import bacc, masks, mybir
from concourse.bass_utils import run_bass_kernel_spmd

N = 8192          # full matrix side
B = 512           # block side
NB = N // B       # 16 blocks per side
P = 128           # SBUF partitions
T = B // P        # 4 row-subtiles per block
GRP = 256         # softmax group width
NG = B // GRP     # 2 groups per block side
TAU = 0.1
NDIAG = 2         # diagonal pairs per core (the last NDIAG slots)
NSLOTS = 17       # block-pairs per core
NOFF = NSLOTS - NDIAG
NCORES = 8
MASK = -60000.0   # pre-masked diagonal value (fp16-representable; exp->0)

F32 = mybir.dt.float32
F16 = mybir.dt.float16
BF16 = mybir.dt.bfloat16

AF = mybir.ActivationFunctionType
OP = mybir.AluOpType


def core_pairs() -> list[list[tuple[int, int]]]:
    """136 upper-triangle block pairs distributed 17-per-core; the 2 diagonal
    pairs of each core come last (the kernel treats those slots specially)."""
    diag = [(i, i) for i in range(NB)]
    off = [(i, j) for i in range(NB) for j in range(i + 1, NB)]
    cps: list[list[tuple[int, int]]] = [[] for _ in range(NCORES)]
    for idx, p in enumerate(off):
        cps[idx % NCORES].append(p)
    for idx, p in enumerate(diag):
        cps[idx % NCORES].append(p)
    return cps


CORE_PAIRS = core_pairs()


def build():
    """Build + compile the (single-program, 8-core SPMD) Bass kernel."""
    nc = bacc.Bacc(
        "TRN2",
        target_bir_lowering=False,
        debug=False,
        enable_asserts=True,
        num_devices=NCORES,
    )
    ab = nc.dram_tensor("ab", [NOFF, P, 2, T, B], F16, kind="ExternalInput").ap()
    ad = nc.dram_tensor("ad", [NDIAG, P, T, B], F16, kind="ExternalInput").ap()
    o = nc.dram_tensor("o", [NSLOTS, P, T, B], F16, kind="ExternalOutput").ap()

    with tile.TileContext(nc) as tc, ExitStack() as ctx:
        const_pool = ctx.enter_context(tc.tile_pool(name="const", bufs=1))
        ident = const_pool.tile([P, P], BF16)
        masks.make_identity(nc, ident[:])
        # All-ones stationary: one matmul both colsums zbt's partition groups
        # AND broadcasts the result to all 128 PSUM partitions. bf16 so the
        # matmuls run in one pass (fp32 matmul = 2 passes).
        ones_mat = const_pool.tile([P, P], BF16)
        nc.gpsimd.memset(ones_mat[:], 1.0)

        ab_pool = ctx.enter_context(tc.tile_pool(name="ab_sb", bufs=5))
        ad_pool = ctx.enter_context(tc.tile_pool(name="ad_sb", bufs=2))
        za_pool = ctx.enter_context(tc.tile_pool(name="za", bufs=4))
        zb_pool = ctx.enter_context(tc.tile_pool(name="zbt", bufs=4))
        x_pool = ctx.enter_context(tc.tile_pool(name="x", bufs=4))
        wa_pool = ctx.enter_context(tc.tile_pool(name="wa", bufs=4))
        sc_pool = ctx.enter_context(tc.tile_pool(name="sc", bufs=2))
        o_pool = ctx.enter_context(tc.tile_pool(name="o_sb", bufs=4))
        st_pool = ctx.enter_context(tc.tile_pool(name="st", bufs=10))
        rp_pool = ctx.enter_context(tc.tile_pool(name="rp", bufs=4))
        dg_pool = ctx.enter_context(tc.tile_pool(name="dg", bufs=2))
        ps_pool = ctx.enter_context(tc.tile_pool(name="ps", bufs=3, space="PSUM"))

        # Diagonal slots are interleaved mid-program: their short chains give
        # ACT/DVE low-dependency filler work between full off-slot chains.
        order = [*range(0, 7), NOFF, *range(7, 12), NOFF + 1, *range(12, NOFF)]
        # Stores are deferred one slot so the store never sits at its queue
        # head waiting on the producing slot's final product.
        pending_store = None
        for k in order:
            diag_slot = k >= NOFF
            if not diag_slot:
                # --- off-diagonal pair: A and host-pre-transposed B ---
                ab_sb = ab_pool.tile([P, 2, T, B], F16)
                nc.sync.dma_start(ab_sb[:], ab[k])

                # BT side: exp (bf16, split in two so the PE can start after
                # the first half); ones-matmuls sum each 256-row partition
                # group into PSUM already broadcast across all partitions.
                zbt = zb_pool.tile([P, T, B], BF16)
                s_ps = ps_pool.tile([P, NG, B], F32, name="p23")
                for g in range(NG):
                    nc.scalar.activation(zbt[:, NG * g:NG * (g + 1), :],
                                         ab_sb[:, 1, NG * g:NG * (g + 1), :],
                                         AF.Exp, scale=1.0 / TAU)
                    for u in range(NG):
                        nc.tensor.matmul(
                            s_ps[:, g, :], ones_mat[:], zbt[:, g * NG + u, :],
                            start=(u == 0), stop=(u == NG - 1),
                        )
                # rp = 1/colsums to SBUF f32 (GpSimd consumes them; it cannot
                # read PSUM). ~51 ULP is plenty.
                rp_sb = rp_pool.tile([P, NG, B], F32)
                for g in range(NG):
                    nc.vector.reciprocal_approx_fast(rp_sb[:, g, :],
                                                     s_ps[:, g, :])

                # A side: 2 big exps, then per-(t,g) sums via 4x-mode
                # tensor_scalar copy+accum into sa (the bf16 copy itself is
                # a scratch dump).
                za = za_pool.tile([P, T, B], BF16)
                sa = st_pool.tile([P, T, NG], F32, name="sa")
                ra = st_pool.tile([P, T, NG], F32, name="ra")
                sc = sc_pool.tile([P, T, B], BF16)
                for h in range(NG):
                    ts = slice(NG * h, NG * (h + 1))
                    nc.scalar.activation(za[:, ts, :], ab_sb[:, 0, ts, :],
                                         AF.Exp, scale=1.0 / TAU)
                for t in range(T):
                    for g in range(NG):
                        cs = slice(g * GRP, (g + 1) * GRP)
                        nc.vector.tensor_scalar(
                            sc[:, t, cs], za[:, t, cs], 1.0, None,
                            op0=OP.mult, op1=OP.add,
                            accum_out=sa[:, t, g:g + 1],
                        )
                nc.vector.reciprocal(ra[:].rearrange("p t g -> p (t g)"),
                                     sa[:].rearrange("p t g -> p (t g)"))

                # X = bhB.T in fp16, fully on GpSimd (2 ops, rp broadcast
                # across the t-pair).
                x_sb = x_pool.tile([P, T, B], F16)
                for h in range(NG):
                    ts = slice(NG * h, NG * (h + 1))
                    nc.gpsimd.tensor_mul(
                        x_sb[:, ts, :], zbt[:, ts, :],
                        rp_sb[:, h:h + 1, :].broadcast_to([P, NG, B]),
                    )

                # wa = za*ra via 4x-mode tensor_scalar_mul, then ONE 2x-mode
                # tensor_tensor for out = wa*X (all operands 16-bit packed).
                wa = wa_pool.tile([P, T, B], BF16)
                o_sb = o_pool.tile([P, T, B], F16)
                for t in range(T):
                    for g in range(NG):
                        cs = slice(g * GRP, (g + 1) * GRP)
                        nc.vector.tensor_scalar_mul(
                            wa[:, t, cs], za[:, t, cs], ra[:, t, g:g + 1])
                nc.vector.tensor_tensor(o_sb[:], wa[:], x_sb[:], op=OP.mult)
            else:
                # --- diagonal pair: B == A, PE bf16 transpose ---
                a_sb = ad_pool.tile([P, T, B], F16)
                nc.sync.dma_start(a_sb[:], ad[k - NOFF])
                za = za_pool.tile([P, T, B], BF16)
                sa = st_pool.tile([P, T, NG], F32, name="sa")
                ra = st_pool.tile([P, T, NG], F32, name="ra")
                sc = sc_pool.tile([P, T, B], BF16)
                for h in range(NG):
                    ts = slice(NG * h, NG * (h + 1))
                    nc.scalar.activation(za[:, ts, :], a_sb[:, ts, :],
                                         AF.Exp, scale=1.0 / TAU)
                for t in range(T):
                    for g in range(NG):
                        cs = slice(g * GRP, (g + 1) * GRP)
                        nc.vector.tensor_scalar(
                            sc[:, t, cs], za[:, t, cs], 1.0, None,
                            op0=OP.mult, op1=OP.add,
                            accum_out=sa[:, t, g:g + 1],
                        )
                nc.vector.reciprocal(ra[:].rearrange("p t g -> p (t g)"),
                                     sa[:].rearrange("p t g -> p (t g)"))
                wa = wa_pool.tile([P, T, B], BF16)
                for t in range(T):
                    for g in range(NG):
                        cs = slice(g * GRP, (g + 1) * GRP)
                        nc.vector.tensor_scalar_mul(
                            wa[:, t, cs], za[:, t, cs], ra[:, t, g:g + 1])
                dg = dg_pool.tile([P, T * NG, P], BF16)
                nc.gpsimd.tensor_mul(
                    dg[:],
                    ident[:].rearrange("p (one c) -> p one c", one=1)
                    .broadcast_to([P, T * NG, P]),
                    ra[:].rearrange("p t g -> p (t g)")
                    .rearrange("p (tg one) -> p tg one", one=1)
                    .broadcast_to([P, T * NG, P]),
                )
                # Two v-waves through one 2-bank PSUM tile; wave 2 reuses the
                # banks after wave 1's products are read.
                p23 = ps_pool.tile([P, NG, B], F32, name="p23")
                o_sb = o_pool.tile([P, T, B], F16)
                for w in range(NG):
                    for hv in range(NG):
                        v = w * NG + hv
                        for u in range(T):
                            nc.tensor.matmul(
                                p23[:, hv, u * P:(u + 1) * P],
                                za[:, u, v * P:(v + 1) * P],
                                dg[:, u * NG + (v // NG), :],
                            )
                        nc.vector.tensor_tensor(
                            o_sb[:, v, :], wa[:, v, :], p23[:, hv, :],
                            op=OP.mult)

            # One whole-block store per slot, alternating between the ACT
            # and GpSimd DMA queues (the two engines closest to balanced;
            # neither queue backs up behind the loads on the sync ring).
            if pending_store is not None:
                eng = nc.scalar if (pending_store[0] % 2 == 0) else nc.gpsimd
                eng.dma_start(o[pending_store[0]], pending_store[1][:])
            pending_store = (k, o_sb)
        eng = nc.scalar if (pending_store[0] % 2 == 0) else nc.gpsimd
        eng.dma_start(o[pending_store[0]], pending_store[1][:])

    nc.compile()
    return nc


_NC = None


def _get_nc():
    global _NC
    if _NC is None:
        _NC = build()
    return _NC


def _to_pmajor(block: np.ndarray) -> np.ndarray:
    # (512, 512) row-major -> (128, 4, 512): row r = t*P + p lands at
    # [p, t, :], so every SBUF partition's bytes are contiguous in DRAM.
    return block.reshape(T, P, B).transpose(1, 0, 2)


def make_in_maps(sims: np.ndarray) -> list[dict[str, np.ndarray]]:
    in_maps = []
    for c in range(NCORES):
        ab_stack = np.empty((NOFF, P, 2, T, B), np.float16)
        ad_stack = np.empty((NDIAG, P, T, B), np.float16)
        for k, (i, j) in enumerate(CORE_PAIRS[c]):
            if k < NOFF:
                assert i != j
                ab_stack[k, :, 0] = _to_pmajor(
                    sims[i * B:(i + 1) * B, j * B:(j + 1) * B]).astype(
                        np.float16)
                ab_stack[k, :, 1] = _to_pmajor(
                    np.ascontiguousarray(
                        sims[j * B:(j + 1) * B, i * B:(i + 1) * B].T)).astype(
                            np.float16)
            else:
                assert i == j
                a = sims[i * B:(i + 1) * B, i * B:(i + 1) * B].copy()
                np.fill_diagonal(a, MASK)
                ad_stack[k - NOFF] = _to_pmajor(a).astype(np.float16)
        in_maps.append({"ab": ab_stack, "ad": ad_stack})
    return in_maps


def assemble(results: list[dict[str, np.ndarray]]) -> np.ndarray:
    out = np.empty((N, N), np.float32)
    for c in range(NCORES):
        o_pm = results[c]["o"]  # (NSLOTS, P, T, B) fp16, partition-major
        o_stack = o_pm.astype(np.float32).transpose(0, 2, 1, 3).reshape(
            NSLOTS, B, B)
        for k, (i, j) in enumerate(CORE_PAIRS[c]):
            out[i * B:(i + 1) * B, j * B:(j + 1) * B] = o_stack[k]
            if i != j:
                out[j * B:(j + 1) * B, i * B:(i + 1) * B] = o_stack[k].T
    return out


def run_on_hw(sims: np.ndarray, **spmd_kwargs):
    """Run the kernel on the 8 NeuronCores. Returns (out, BassKernelResults).

    The device occasionally throws a transient NRT_EXEC_UNIT_UNRECOVERABLE
    and needs ~a minute to come back, so failed runs are retried."""
    import time

    nc = _get_nc()
    in_maps = make_in_maps(sims)
    last_exc = None
    for attempt in range(3):
        if attempt:
            time.sleep(75)
        try:
            res = run_bass_kernel_spmd(
                nc, in_maps, core_ids=list(range(NCORES)), **spmd_kwargs
            )
            return assemble(res.results), res
        except Exception as exc:  # noqa: BLE001 - device flake, retry
            last_exc = exc
    raise last_exc


def kernel(similarities: np.ndarray) -> np.ndarray:
    sims = np.ascontiguousarray(similarities, dtype=np.float32)
    assert sims.shape == (N, N)
    out, _ = run_on_hw(sims)
    return out


if __name__ == "__main__":
    rng = np.random.default_rng(0)
    sims = rng.standard_normal((N, N), dtype=np.float32)
    out = kernel(similarities=sims)
    print("out", out.shape, out.dtype, float(out.max()))
